# revision 40
# baseline (speedup 1.0000x reference)
# kernel.py — Trainium2 Bass kernel for nn_MockQwenForCausalLM (loss_fn)
#
#   hidden = embed[input_ids]            [B*S, H]
#   logits = hidden @ W.T                [B*S, V]   (returned)
#   loss   = shifted cross-entropy       scalar     (returned)
#
# Strategy (8 cores, vocab-sharded tensor parallel):
#   - each core holds W^T for a 4000-wide vocab shard (bf16, host-pre-transposed)
#   - each core gathers the full hidden activations from its replica of the
#     (bf16) embedding table via indirect DMA, transposes 128x128 tiles on the
#     tensor engine to get hidden^T (contraction dim on partitions)
#   - matmul: logits tile [128 tok, 500 vocab] accumulated over 16 k-chunks
#   - fused per-tile: exp+rowsum on ACT (accum_out), label-logit pick on DVE
#     (iota==label mask via scalar_tensor_tensor with accum_out), logits
#     copy-out PSUM->SBUF->DRAM
#   - AllReduce (8 cores) of per-row [sumexp, label-logit], then each core
#     computes the final scalar loss on-device
#
# Self-contained: hardcodes all shapes; imports only installed packages.

import math
import os

import numpy as np

P = 128

# ---- full problem config (hardcoded from the task spec) ----
FULL_CFG = dict(
    B=4,
    S=2048,
    V=32000,
    H=2048,
    n_cores=8,
    NT=500,  # vocab tile (<= 512, divides V // n_cores)
    out_dt="bfloat16",  # logits output dtype ("float32" | "bfloat16")
    # NOTE: loading the NEFF with collectives enabled caps the PE clock at
    # ~1.95 GHz (vs 2.4 GHz without) on this platform — a ~15% kernel-wide
    # penalty — so the final 8-way [128,128] stats sum happens on the host
    # during unsharding instead (the per-shard sumexp/label reductions all
    # stay on device).
    use_collective=False,
    tp_mode="dma",  # hidden^T transpose path: "pe" (TensorE) | "dma" (xbar)
    cc_split=8,  # trailing m-blocks reduced in the (small) tail AllReduce
)

_BUILD_CACHE = {}


def _dt(mybir, name):
    return {"float32": mybir.dt.float32, "bfloat16": mybir.dt.bfloat16}[name]


def build_bass(cfg):
    """Build + compile the per-core Bass program. Returns the Bacc object."""
    key = tuple(sorted(cfg.items()))
    if key in _BUILD_CACHE:
        return _BUILD_CACHE[key]

    import concourse.bass as bass
    import concourse.tile as tile
    from concourse import bacc, mybir
    from concourse.bass import ts
    from concourse.masks import make_identity

    B, S, V, H = cfg["B"], cfg["S"], cfg["V"], cfg["H"]
    n_cores, NT = cfg["n_cores"], cfg["NT"]
    T = B * S
    VS = V // n_cores
    KC = H // P
    MB = T // P
    NB = VS // NT
    assert VS % NT == 0 and H % P == 0 and T % P == 0
    out_dt = _dt(mybir, cfg["out_dt"])
    f32 = mybir.dt.float32
    bf16 = mybir.dt.bfloat16
    use_cc = cfg["use_collective"]

    nc = bacc.Bacc(
        "TRN2",
        target_bir_lowering=False,
        debug=False,
        enable_asserts=False,
        num_devices=n_cores,
    )

    NQ = 4 if VS % (4 * NT) == 0 else 1  # contiguous WT load chunks
    QV = VS // NQ

    embed_d = nc.dram_tensor("embed", [V, H], bf16, kind="ExternalInput")
    # host-retiled W^T: wt[p, q, k, v] = W^T[k*128+p, q*QV+v]
    wt_d = nc.dram_tensor("wt", [P, NQ, KC, QV], bf16, kind="ExternalInput")
    ids_d = nc.dram_tensor("ids", [P, MB], mybir.dt.int32, kind="ExternalInput")
    labrel_d = nc.dram_tensor("labrel", [P, MB], f32, kind="ExternalInput")
    wmask_d = nc.dram_tensor("wmask", [P, MB], f32, kind="ExternalInput")
    logits_d = nc.dram_tensor("logits", [T, VS], out_dt, kind="ExternalOutput")
    loss_d = nc.dram_tensor("loss", [1, 1], f32, kind="ExternalOutput")
    if not use_cc:
        stats_d = nc.dram_tensor("stats", [P, 2 * MB], f32, kind="ExternalOutput")

    with tile.TileContext(nc) as tc:
        with (
            tc.tile_pool(name="const", bufs=1) as constp,
            tc.tile_pool(name="wtp", bufs=1) as wtp,
            tc.tile_pool(name="gath", bufs=6) as gathp,
            tc.tile_pool(name="hT", bufs=6) as hTp,
            tc.tile_pool(name="louts", bufs=6) as loutp,
            tc.tile_pool(name="trash", bufs=2) as trashp,
            tc.tile_pool(name="parts", bufs=3) as partsp,
            tc.tile_pool(name="stats", bufs=1) as statsp,
            tc.tile_pool(name="psmm", bufs=4, space="PSUM") as psmm,
            tc.tile_pool(name="pstp", bufs=2, space="PSUM") as pstp,
            tc.tile_pool(name="psls", bufs=1, space="PSUM") as psls,
            tc.tile_pool(name="dram", bufs=1, space="DRAM") as dramp,
        ):
            # ---- constants ----
            tp_dma = cfg["tp_mode"] == "dma"
            if not tp_dma:
                ident = constp.tile([P, P], bf16)
                make_identity(nc, ident[:])
            iota_i = constp.tile([P, NT], mybir.dt.int32)
            nc.gpsimd.iota(iota_i[:], pattern=[[1, NT]], base=0, channel_multiplier=0)
            iota_f = constp.tile([P, NT], f32)
            nc.vector.tensor_copy(iota_f[:], iota_i[:])
            noff_i = constp.tile([P, NB], mybir.dt.int32)
            nc.gpsimd.iota(noff_i[:], pattern=[[NT, NB]], base=0, channel_multiplier=0)
            noff_f = constp.tile([P, NB], f32)
            nc.vector.tensor_copy(noff_f[:], noff_i[:])
            ones_t = constp.tile([P, 1], f32)
            nc.vector.memset(ones_t[:], 1.0)

            ids_sb = constp.tile([P, MB], mybir.dt.int32)
            nc.sync.dma_start(ids_sb[:], ids_d.ap())

            # tiny dummy indirect gather to absorb the one-time software-DGE
            # queue warmup (~25us) before the first real gather needs it
            zoff = constp.tile([P, 1], mybir.dt.int32)
            nc.vector.memset(zoff[:], 0)
            dummy = constp.tile([P, 16], bf16)
            nc.gpsimd.indirect_dma_start(
                out=dummy[:],
                out_offset=None,
                in_=embed_d.ap()[:, 0:16],
                in_offset=bass.IndirectOffsetOnAxis(ap=zoff[:, :1], axis=0),
            )
            labrel_sb = constp.tile([P, MB], f32)
            nc.scalar.dma_start(labrel_sb[:], labrel_d.ap())
            wmask_sb = constp.tile([P, MB], f32)
            nc.scalar.dma_start(wmask_sb[:], wmask_d.ap())

            # ---- gather + transpose pipeline (issued ahead of WT so the
            # first token blocks drain the slow indirect queue early) ----
            PRE = 5
            hT_q = {}
            gather_inst = {}

            def emit_gather(m):
                gath = gathp.tile([P, H], bf16, name="gath")
                gi = nc.gpsimd.indirect_dma_start(
                    out=gath[:],
                    out_offset=None,
                    in_=embed_d.ap(),
                    in_offset=bass.IndirectOffsetOnAxis(
                        ap=ids_sb[:, m : m + 1], axis=0
                    ),
                )
                gather_inst[m] = gi
                # xbar semantics: out[p, j, t] = in[t, j*128 + p]
                hT = hTp.tile([P, KC, P], bf16, name="hT")
                if tp_dma:
                    nc.sync.dma_start_transpose(out=hT[:], in_=gath[:])
                else:
                    for k in range(KC):
                        tp_ps = pstp.tile([P, P], bf16, space="PSUM", name="tp")
                        nc.tensor.transpose(tp_ps[:], gath[:, ts(k, P)], ident[:])
                        nc.vector.tensor_copy(hT[:, k, :], tp_ps[:])
                hT_q[m] = hT

            for m in range(min(PRE, MB)):
                emit_gather(m)

            # ---- resident W^T [P, NQ, KC, QV] bf16, loaded as NQ fully
            # contiguous DMAs so early vocab chunks land before the first
            # matmul sweep without clogging the trigger queue ----
            wt_sb = wtp.tile([P, NQ, KC, QV], bf16)
            prev_wt = None
            for q in range(NQ):
                wi = nc.scalar.dma_start(wt_sb[:, q], wt_d.ap()[:, q])
                if q == 0 and min(PRE, MB) >= 3:
                    # let the first gathers drain before the big WT stream
                    # grabs the HBM/SDMA bandwidth
                    tile.add_dep_helper(
                        wi.ins,
                        gather_inst[2].ins,
                        sync=True,
                        reason="gathers-before-wt",
                    )
                if prev_wt is not None:
                    # chain so early quarters finish first (otherwise the
                    # SDMA engines round-robin all quarters and the first
                    # completes only at the very end)
                    tile.add_dep_helper(
                        wi.ins, prev_wt.ins, sync=True, reason="wt-load-order"
                    )
                prev_wt = wi

            # ---- per-row statistic accumulators ----
            S_all = statsp.tile([P, MB], f32)
            T_all = statsp.tile([P, MB], f32)

            M1 = max(1, MB - cfg["cc_split"]) if use_cc else MB
            if use_cc:
                cc1_in = dramp.tile([P, 2 * M1], f32)
                cc1_out = dramp.tile([P, 2 * M1], f32, addr_space="Shared")
                M2 = MB - M1
                cc2_in = dramp.tile([P, 2 * M2], f32)
                cc2_out = dramp.tile([P, 2 * M2], f32, addr_space="Shared")

            for m in range(MB):
                if m + PRE < MB:
                    emit_gather(m + PRE)
                hT = hT_q.pop(m)

                # labrel for each n-block: labrel - n*NT
                lr8 = partsp.tile([P, NB], f32, name="lr8")
                nc.vector.tensor_tensor(
                    out=lr8[:],
                    in0=labrel_sb[:, m : m + 1].to_broadcast([P, NB]),
                    in1=noff_f[:],
                    op=mybir.AluOpType.subtract,
                )
                se8 = partsp.tile([P, NB], f32, name="se8")
                lp8 = partsp.tile([P, NB], f32, name="lp8")

                for n in range(NB):
                    ps = psmm.tile([P, NT], f32, space="PSUM", name="ps")
                    q, hh = (n * NT) // QV, (n * NT) % QV
                    for k in range(KC):
                        nc.tensor.matmul(
                            ps[:],
                            lhsT=hT[:, k, :],
                            rhs=wt_sb[:, q, k, hh : hh + NT],
                            start=(k == 0),
                            stop=(k == KC - 1),
                        )
                    # logits copy-out first: the cast is the ONLY psum reader,
                    # so the bank recycles as soon as it completes (the psum
                    # slot free was pacing the next block's first matmul)
                    lo = loutp.tile([P, NT], out_dt, name="lo")
                    nc.vector.tensor_copy(lo[:], ps[:])
                    nc.scalar.dma_start(logits_d.ap()[ts(m, P), ts(n, NT)], lo[:])
                    # exp + row-sum (ACT) from the bf16 copy
                    trash_e = trashp.tile([P, NT], f32, name="trash_e")
                    nc.scalar.activation(
                        out=trash_e[:],
                        in_=lo[:],
                        func=mybir.ActivationFunctionType.Exp,
                        accum_out=se8[:, n : n + 1],
                    )
                    # label-logit pick (DVE): sum((iota == labrel) * logits)
                    trash_l = trashp.tile([P, NT], f32, name="trash_l")
                    nc.vector.scalar_tensor_tensor(
                        out=trash_l[:],
                        in0=iota_f[:],
                        scalar=lr8[:, n : n + 1],
                        in1=lo[:],
                        op0=mybir.AluOpType.is_equal,
                        op1=mybir.AluOpType.mult,
                        accum_out=lp8[:, n : n + 1],
                    )

                nc.vector.reduce_sum(
                    out=S_all[:, m : m + 1], in_=se8[:], axis=mybir.AxisListType.X
                )
                nc.vector.reduce_sum(
                    out=T_all[:, m : m + 1], in_=lp8[:], axis=mybir.AxisListType.X
                )

                if use_cc and m == M1 - 1:
                    # bulk AllReduce for the first M1 blocks, hidden under the
                    # remaining compute (gpsimd ring: keep ACT's FIFO free)
                    nc.gpsimd.dma_start(cc1_in[:, 0:M1], S_all[:, 0:M1])
                    nc.gpsimd.dma_start(cc1_in[:, M1 : 2 * M1], T_all[:, 0:M1])
                    nc.gpsimd.collective_compute(
                        "AllReduce",
                        mybir.AluOpType.add,
                        replica_groups=[list(range(n_cores))],
                        ins=[cc1_in[:].opt()],
                        outs=[cc1_out[:].opt()],
                    )

            # ---- cross-core reduction + final loss ----
            if use_cc:
                red = statsp.tile([P, 2 * MB], f32)
                if M2 > 0:
                    nc.gpsimd.dma_start(cc2_in[:, 0:M2], S_all[:, M1:MB])
                    nc.gpsimd.dma_start(cc2_in[:, M2 : 2 * M2], T_all[:, M1:MB])
                    nc.gpsimd.collective_compute(
                        "AllReduce",
                        mybir.AluOpType.add,
                        replica_groups=[list(range(n_cores))],
                        ins=[cc2_in[:].opt()],
                        outs=[cc2_out[:].opt()],
                    )
                    nc.scalar.dma_start(red[:, M1:MB], cc2_out[:, 0:M2])
                    nc.scalar.dma_start(red[:, MB + M1 : 2 * MB], cc2_out[:, M2 : 2 * M2])
                nc.scalar.dma_start(red[:, 0:M1], cc1_out[:, 0:M1])
                nc.scalar.dma_start(red[:, MB : MB + M1], cc1_out[:, M1 : 2 * M1])
                logS = statsp.tile([P, MB], f32)
                nc.scalar.activation(
                    out=logS[:], in_=red[:, 0:MB], func=mybir.ActivationFunctionType.Ln
                )
                dif = statsp.tile([P, MB], f32)
                nc.vector.tensor_tensor(
                    out=dif[:],
                    in0=logS[:],
                    in1=red[:, MB : 2 * MB],
                    op=mybir.AluOpType.subtract,
                )
                dif2 = statsp.tile([P, MB], f32)
                fvec = statsp.tile([P, 1], f32)
                nc.vector.scalar_tensor_tensor(
                    out=dif2[:],
                    in0=dif[:],
                    scalar=1.0,
                    in1=wmask_sb[:],
                    op0=mybir.AluOpType.mult,
                    op1=mybir.AluOpType.mult,
                    accum_out=fvec[:],
                )
                ls_ps = psls.tile([1, 1], f32, space="PSUM", name="lsps")
                nc.tensor.matmul(
                    ls_ps[:], lhsT=fvec[:], rhs=ones_t[:], start=True, stop=True
                )
                ls_sb = statsp.tile([1, 1], f32)
                nc.vector.tensor_copy(ls_sb[:], ls_ps[:])
                nc.scalar.dma_start(loss_d.ap(), ls_sb[:])
            else:
                # export per-core stats; host reduces
                nc.scalar.dma_start(stats_d.ap()[:, 0:MB], S_all[:])
                nc.scalar.dma_start(stats_d.ap()[:, MB : 2 * MB], T_all[:])
                zz = statsp.tile([1, 1], f32)
                nc.vector.memset(zz[:], 0.0)
                nc.scalar.dma_start(loss_d.ap(), zz[:])

    nc.compile()
    _BUILD_CACHE[key] = nc
    return nc


def prep_inputs(cfg, input_ids, labels, embed, W):
    """Host-side sharding/layout prep. Returns (in_maps, denom)."""
    import ml_dtypes

    B, S, V, H = cfg["B"], cfg["S"], cfg["V"], cfg["H"]
    n_cores = cfg["n_cores"]
    T = B * S
    VS = V // n_cores
    MB = T // P
    IGNORE_INDEX = -100

    ids = np.ascontiguousarray(
        np.asarray(input_ids).reshape(T).astype(np.int32).reshape(MB, P).T
    )

    lab = np.asarray(labels).reshape(B, S)
    labshift = np.full((B, S), -3.0e9, np.float32)
    nxt = lab[:, 1:]
    valid = nxt != IGNORE_INDEX
    labshift[:, :-1] = np.where(valid, nxt.astype(np.float32), -3.0e9)
    denom = max(int(valid.sum()), 1)
    labflat = labshift.reshape(T)

    wm = np.zeros((B, S), np.float32)
    wm[:, :-1] = valid.astype(np.float32) / denom
    wm_arr = np.ascontiguousarray(wm.reshape(T).reshape(MB, P).T)

    embed_bf = np.asarray(embed, np.float32).astype(ml_dtypes.bfloat16)

    VS = V // n_cores
    NT = cfg["NT"]
    KC = H // P
    NQ = 4 if VS % (4 * NT) == 0 else 1
    QV = VS // NQ

    in_maps = []
    for c in range(n_cores):
        v0 = c * VS
        # wt[p, q, k, v] = W^T[k*128+p, q*QV+v] = W[v0+q*QV+v, k*128+p]
        wt_t = np.asarray(W, np.float32)[v0 : v0 + VS].T.astype(ml_dtypes.bfloat16)
        wt_c = np.ascontiguousarray(
            wt_t.reshape(KC, P, NQ, QV).transpose(1, 2, 0, 3)
        )
        labrel_c = np.ascontiguousarray(
            (labflat - np.float32(v0)).reshape(MB, P).T.astype(np.float32)
        )
        in_maps.append(
            {
                "embed": embed_bf,
                "wt": wt_c,
                "ids": ids,
                "labrel": labrel_c,
                "wmask": wm_arr,
            }
        )
    return in_maps, {"denom": denom, "wmask": wm_arr}


def assemble_outputs(cfg, results, extras):
    """Combine per-core outputs into (loss, logits)."""
    B, S, V = cfg["B"], cfg["S"], cfg["V"]
    n_cores = cfg["n_cores"]
    T = B * S
    MB = T // P
    logits = np.concatenate(
        [np.asarray(results[c]["logits"], np.float32) for c in range(n_cores)], axis=1
    ).reshape(B, S, V)
    if cfg["use_collective"]:
        loss = np.float32(np.asarray(results[0]["loss"]).reshape(-1)[0])
    else:
        # final 8-way sum of the per-shard [sumexp, label-logit] stats
        S_sum = np.zeros((P, MB), np.float64)
        T_sum = np.zeros((P, MB), np.float64)
        for c in range(n_cores):
            st = np.asarray(results[c]["stats"], np.float64)
            S_sum += st[:, 0:MB]
            T_sum += st[:, MB:]
        valid = extras["wmask"] > 0
        loss = np.float32(
            float(((np.log(S_sum) - T_sum) * valid).sum() / extras["denom"])
        )
    return loss, logits


def run_on_hw(cfg, in_maps, trace=False, **kw):
    from concourse import bass_utils

    nc = build_bass(cfg)
    res = bass_utils.run_bass_kernel_spmd(
        nc, in_maps, core_ids=list(range(cfg["n_cores"])), trace=trace, **kw
    )
    return res


def kernel(input_ids, labels, embed, W):
    cfg = dict(FULL_CFG)
    in_maps, denom = prep_inputs(cfg, input_ids, labels, embed, W)
    res = run_on_hw(cfg, in_maps, trace=False)
    return assemble_outputs(cfg, res.results, denom)


# revision 43
# speedup vs baseline: 1.2034x; 1.2034x over previous
# kernel.py — Trainium2 Bass kernel for nn_MockQwenForCausalLM (loss_fn)
#
#   hidden = embed[input_ids]            [B*S, H]
#   logits = hidden @ W.T                [B*S, V]   (returned)
#   loss   = shifted cross-entropy       scalar     (returned)
#
# Strategy (8 cores, vocab-sharded tensor parallel):
#   - each core holds W^T for a 4000-wide vocab shard (bf16, host-pre-transposed)
#   - each core gathers the full hidden activations from its replica of the
#     (bf16) embedding table via indirect DMA, transposes 128x128 tiles on the
#     tensor engine to get hidden^T (contraction dim on partitions)
#   - matmul: logits tile [128 tok, 500 vocab] accumulated over 16 k-chunks
#   - fused per-tile: exp+rowsum on ACT (accum_out), label-logit pick on DVE
#     (iota==label mask via scalar_tensor_tensor with accum_out), logits
#     copy-out PSUM->SBUF->DRAM
#   - AllReduce (8 cores) of per-row [sumexp, label-logit], then each core
#     computes the final scalar loss on-device
#
# Self-contained: hardcodes all shapes; imports only installed packages.

import math
import os

import numpy as np

P = 128

# ---- full problem config (hardcoded from the task spec) ----
FULL_CFG = dict(
    B=4,
    S=2048,
    V=32000,
    H=2048,
    n_cores=8,
    NT=500,  # vocab tile (<= 512, divides V // n_cores)
    out_dt="bfloat16",  # logits output dtype ("float32" | "bfloat16")
    # NOTE: loading the NEFF with collectives enabled caps the PE clock at
    # ~1.95 GHz (vs 2.4 GHz without) on this platform — a ~15% kernel-wide
    # penalty — so the final 8-way [128,128] stats sum happens on the host
    # during unsharding instead (the per-shard sumexp/label reductions all
    # stay on device).
    use_collective=False,
    tp_mode="dma",  # hidden^T transpose path: "pe" (TensorE) | "dma" (xbar)
    cc_split=8,  # trailing m-blocks reduced in the (small) tail AllReduce
)

_BUILD_CACHE = {}


def _dt(mybir, name):
    return {"float32": mybir.dt.float32, "bfloat16": mybir.dt.bfloat16}[name]


def build_bass(cfg):
    """Build + compile the per-core Bass program. Returns the Bacc object."""
    key = tuple(sorted(cfg.items()))
    if key in _BUILD_CACHE:
        return _BUILD_CACHE[key]

    import concourse.bass as bass
    import concourse.tile as tile
    from concourse import bacc, mybir
    from concourse.bass import ts
    from concourse.masks import make_identity

    B, S, V, H = cfg["B"], cfg["S"], cfg["V"], cfg["H"]
    n_cores, NT = cfg["n_cores"], cfg["NT"]
    T = B * S
    VS = V // n_cores
    KC = H // P
    MB = T // P
    NB = VS // NT
    assert VS % NT == 0 and H % P == 0 and T % P == 0
    out_dt = _dt(mybir, cfg["out_dt"])
    f32 = mybir.dt.float32
    bf16 = mybir.dt.bfloat16
    use_cc = cfg["use_collective"]

    nc = bacc.Bacc(
        "TRN2",
        target_bir_lowering=False,
        debug=False,
        enable_asserts=False,
        num_devices=n_cores,
    )

    NQ = 4 if VS % (4 * NT) == 0 else 1  # contiguous WT load chunks
    QV = VS // NQ

    embed_d = nc.dram_tensor("embed", [V, H], bf16, kind="ExternalInput")
    # host-retiled W^T: wt[p, q, k, v] = W^T[k*128+p, q*QV+v]
    wt_d = nc.dram_tensor("wt", [P, NQ, KC, QV], bf16, kind="ExternalInput")
    ids_d = nc.dram_tensor("ids", [P, MB], mybir.dt.int32, kind="ExternalInput")
    labrel_d = nc.dram_tensor("labrel", [P, MB], f32, kind="ExternalInput")
    wmask_d = nc.dram_tensor("wmask", [P, MB], f32, kind="ExternalInput")
    logits_d = nc.dram_tensor("logits", [T, VS], out_dt, kind="ExternalOutput")
    loss_d = nc.dram_tensor("loss", [1, 1], f32, kind="ExternalOutput")
    if not use_cc:
        stats_d = nc.dram_tensor("stats", [P, 2 * MB], f32, kind="ExternalOutput")

    with tile.TileContext(nc) as tc:
        with (
            tc.tile_pool(name="const", bufs=1) as constp,
            tc.tile_pool(name="wtp", bufs=1) as wtp,
            tc.tile_pool(name="gath", bufs=6) as gathp,
            tc.tile_pool(name="hT", bufs=6) as hTp,
            tc.tile_pool(name="louts", bufs=6) as loutp,
            tc.tile_pool(name="trash", bufs=2) as trashp,
            tc.tile_pool(name="parts", bufs=3) as partsp,
            tc.tile_pool(name="stats", bufs=1) as statsp,
            tc.tile_pool(name="psmm", bufs=4, space="PSUM") as psmm,
            tc.tile_pool(name="pstp", bufs=2, space="PSUM") as pstp,
            tc.tile_pool(name="psls", bufs=1, space="PSUM") as psls,
            tc.tile_pool(name="dram", bufs=1, space="DRAM") as dramp,
        ):
            # ---- constants ----
            tp_dma = cfg["tp_mode"] == "dma"
            if not tp_dma:
                ident = constp.tile([P, P], bf16)
                make_identity(nc, ident[:])
            iota_i = constp.tile([P, NT], mybir.dt.int32)
            nc.gpsimd.iota(iota_i[:], pattern=[[1, NT]], base=0, channel_multiplier=0)
            iota_f = constp.tile([P, NT], f32)
            nc.vector.tensor_copy(iota_f[:], iota_i[:])
            noff_i = constp.tile([P, NB], mybir.dt.int32)
            nc.gpsimd.iota(noff_i[:], pattern=[[NT, NB]], base=0, channel_multiplier=0)
            noff_f = constp.tile([P, NB], f32)
            nc.vector.tensor_copy(noff_f[:], noff_i[:])
            ones_t = constp.tile([P, 1], f32)
            nc.vector.memset(ones_t[:], 1.0)

            ids_sb = constp.tile([P, MB], mybir.dt.int32)
            nc.sync.dma_start(ids_sb[:], ids_d.ap())

            # tiny dummy indirect gather to absorb the one-time software-DGE
            # queue warmup (~25us) before the first real gather needs it
            zoff = constp.tile([P, 1], mybir.dt.int32)
            nc.vector.memset(zoff[:], 0)
            dummy = constp.tile([P, 16], bf16)
            nc.gpsimd.indirect_dma_start(
                out=dummy[:],
                out_offset=None,
                in_=embed_d.ap()[:, 0:16],
                in_offset=bass.IndirectOffsetOnAxis(ap=zoff[:, :1], axis=0),
            )
            labrel_sb = constp.tile([P, MB], f32)
            nc.scalar.dma_start(labrel_sb[:], labrel_d.ap())
            wmask_sb = constp.tile([P, MB], f32)
            nc.scalar.dma_start(wmask_sb[:], wmask_d.ap())

            # ---- gather + transpose pipeline (issued ahead of WT so the
            # first token blocks drain the slow indirect queue early) ----
            PRE = 5
            hT_q = {}
            gather_inst = {}

            def emit_gather(m):
                gath = gathp.tile([P, H], bf16, name="gath")
                gi = nc.gpsimd.indirect_dma_start(
                    out=gath[:],
                    out_offset=None,
                    in_=embed_d.ap(),
                    in_offset=bass.IndirectOffsetOnAxis(
                        ap=ids_sb[:, m : m + 1], axis=0
                    ),
                )
                gather_inst[m] = gi
                # xbar semantics: out[p, j, t] = in[t, j*128 + p]
                hT = hTp.tile([P, KC, P], bf16, name="hT")
                if tp_dma:
                    nc.sync.dma_start_transpose(out=hT[:], in_=gath[:])
                else:
                    for k in range(KC):
                        tp_ps = pstp.tile([P, P], bf16, space="PSUM", name="tp")
                        nc.tensor.transpose(tp_ps[:], gath[:, ts(k, P)], ident[:])
                        nc.vector.tensor_copy(hT[:, k, :], tp_ps[:])
                hT_q[m] = hT

            for m in range(min(PRE, MB)):
                emit_gather(m)

            # ---- resident W^T [P, NQ, KC, QV] bf16, loaded as NQ fully
            # contiguous DMAs so early vocab chunks land before the first
            # matmul sweep without clogging the trigger queue ----
            wt_sb = wtp.tile([P, NQ, KC, QV], bf16)
            wt_insts = {}

            def emit_wt(q):
                wi = nc.scalar.dma_start(wt_sb[:, q], wt_d.ap()[:, q])
                if q > 0:
                    # chain so early quarters finish first (otherwise the
                    # SDMA engines round-robin all quarters and the first
                    # completes only at the very end)
                    tile.add_dep_helper(
                        wi.ins, wt_insts[q - 1].ins, sync=True, reason="wt-order"
                    )
                wt_insts[q] = wi

            emit_wt(0)
            # n-groups per WT quarter; blocks interleaved during warm-up so the
            # first sweep never outruns the WT arrival chain
            GPH = max(1, NB // NQ)
            WARM = 3 if (NQ == 4 and MB >= 6) else 0
            if WARM == 0:
                for q in range(1, NQ):
                    emit_wt(q)

            # ---- per-row statistic accumulators ----
            S_all = statsp.tile([P, MB], f32)
            T_all = statsp.tile([P, MB], f32)

            M1 = max(1, MB - cfg["cc_split"]) if use_cc else MB
            if use_cc:
                cc1_in = dramp.tile([P, 2 * M1], f32)
                cc1_out = dramp.tile([P, 2 * M1], f32, addr_space="Shared")
                M2 = MB - M1
                cc2_in = dramp.tile([P, 2 * M2], f32)
                cc2_out = dramp.tile([P, 2 * M2], f32, addr_space="Shared")

            state = {}

            def touch_block(m):
                hT = hT_q.pop(m)
                # labrel for each n-block: labrel - n*NT
                lr8 = partsp.tile([P, NB], f32, name="lr8")
                nc.vector.tensor_tensor(
                    out=lr8[:],
                    in0=labrel_sb[:, m : m + 1].to_broadcast([P, NB]),
                    in1=noff_f[:],
                    op=mybir.AluOpType.subtract,
                )
                se8 = partsp.tile([P, NB], f32, name="se8")
                lp8 = partsp.tile([P, NB], f32, name="lp8")
                state[m] = [hT, lr8, se8, lp8, 0]

            def emit_group(m, n):
                if m not in state:
                    touch_block(m)
                hT, lr8, se8, lp8, cnt = state[m]
                ps = psmm.tile([P, NT], f32, space="PSUM", name="ps")
                q, hh = (n * NT) // QV, (n * NT) % QV
                for k in range(KC):
                    nc.tensor.matmul(
                        ps[:],
                        lhsT=hT[:, k, :],
                        rhs=wt_sb[:, q, k, hh : hh + NT],
                        start=(k == 0),
                        stop=(k == KC - 1),
                    )
                # logits copy-out first: the cast is the ONLY psum reader, so
                # the bank recycles as soon as it completes
                lo = loutp.tile([P, NT], out_dt, name="lo")
                nc.vector.tensor_copy(lo[:], ps[:])
                nc.scalar.dma_start(logits_d.ap()[ts(m, P), ts(n, NT)], lo[:])
                # exp + row-sum (ACT) from the bf16 copy
                trash_e = trashp.tile([P, NT], f32, name="trash_e")
                nc.scalar.activation(
                    out=trash_e[:],
                    in_=lo[:],
                    func=mybir.ActivationFunctionType.Exp,
                    accum_out=se8[:, n : n + 1],
                )
                # label-logit pick (DVE): sum((iota == labrel) * logits)
                trash_l = trashp.tile([P, NT], f32, name="trash_l")
                nc.vector.scalar_tensor_tensor(
                    out=trash_l[:],
                    in0=iota_f[:],
                    scalar=lr8[:, n : n + 1],
                    in1=lo[:],
                    op0=mybir.AluOpType.is_equal,
                    op1=mybir.AluOpType.mult,
                    accum_out=lp8[:, n : n + 1],
                )
                state[m][4] = cnt + 1
                if state[m][4] == NB:
                    finish_block(m)

            def finish_block(m):
                _, _, se8, lp8, _ = state.pop(m)
                nc.vector.reduce_sum(
                    out=S_all[:, m : m + 1], in_=se8[:], axis=mybir.AxisListType.X
                )
                nc.vector.reduce_sum(
                    out=T_all[:, m : m + 1], in_=lp8[:], axis=mybir.AxisListType.X
                )
                if m + PRE < MB:
                    emit_gather(m + PRE)
                if use_cc and m == M1 - 1:
                    # bulk AllReduce for the first M1 blocks, hidden under the
                    # remaining compute (gpsimd ring: keep ACT's FIFO free)
                    nc.gpsimd.dma_start(cc1_in[:, 0:M1], S_all[:, 0:M1])
                    nc.gpsimd.dma_start(cc1_in[:, M1 : 2 * M1], T_all[:, 0:M1])
                    nc.gpsimd.collective_compute(
                        "AllReduce",
                        mybir.AluOpType.add,
                        replica_groups=[list(range(n_cores))],
                        ins=[cc1_in[:].opt()],
                        outs=[cc1_out[:].opt()],
                    )

            # ---- drive the schedule ----
            # warm-up: phase p sweeps n-groups [p*GPH, (p+1)*GPH) of the first
            # WARM blocks; WT quarter p+1's (chained) trigger is emitted at the
            # phase-p boundary so its FIFO wait resolves before the phase needs
            # the ACT queue
            for p in range(NQ if WARM else 0):
                if p + 1 < NQ:
                    emit_wt(p + 1)
                for m in range(WARM):
                    for g in range(GPH):
                        emit_group(m, p * GPH + g)
            for m in range(WARM, MB):
                for n in range(NB):
                    emit_group(m, n)

            # ---- cross-core reduction + final loss ----
            if use_cc:
                red = statsp.tile([P, 2 * MB], f32)
                if M2 > 0:
                    nc.gpsimd.dma_start(cc2_in[:, 0:M2], S_all[:, M1:MB])
                    nc.gpsimd.dma_start(cc2_in[:, M2 : 2 * M2], T_all[:, M1:MB])
                    nc.gpsimd.collective_compute(
                        "AllReduce",
                        mybir.AluOpType.add,
                        replica_groups=[list(range(n_cores))],
                        ins=[cc2_in[:].opt()],
                        outs=[cc2_out[:].opt()],
                    )
                    nc.scalar.dma_start(red[:, M1:MB], cc2_out[:, 0:M2])
                    nc.scalar.dma_start(red[:, MB + M1 : 2 * MB], cc2_out[:, M2 : 2 * M2])
                nc.scalar.dma_start(red[:, 0:M1], cc1_out[:, 0:M1])
                nc.scalar.dma_start(red[:, MB : MB + M1], cc1_out[:, M1 : 2 * M1])
                logS = statsp.tile([P, MB], f32)
                nc.scalar.activation(
                    out=logS[:], in_=red[:, 0:MB], func=mybir.ActivationFunctionType.Ln
                )
                dif = statsp.tile([P, MB], f32)
                nc.vector.tensor_tensor(
                    out=dif[:],
                    in0=logS[:],
                    in1=red[:, MB : 2 * MB],
                    op=mybir.AluOpType.subtract,
                )
                dif2 = statsp.tile([P, MB], f32)
                fvec = statsp.tile([P, 1], f32)
                nc.vector.scalar_tensor_tensor(
                    out=dif2[:],
                    in0=dif[:],
                    scalar=1.0,
                    in1=wmask_sb[:],
                    op0=mybir.AluOpType.mult,
                    op1=mybir.AluOpType.mult,
                    accum_out=fvec[:],
                )
                ls_ps = psls.tile([1, 1], f32, space="PSUM", name="lsps")
                nc.tensor.matmul(
                    ls_ps[:], lhsT=fvec[:], rhs=ones_t[:], start=True, stop=True
                )
                ls_sb = statsp.tile([1, 1], f32)
                nc.vector.tensor_copy(ls_sb[:], ls_ps[:])
                nc.scalar.dma_start(loss_d.ap(), ls_sb[:])
            else:
                # export per-core stats; host reduces
                nc.scalar.dma_start(stats_d.ap()[:, 0:MB], S_all[:])
                nc.scalar.dma_start(stats_d.ap()[:, MB : 2 * MB], T_all[:])
                zz = statsp.tile([1, 1], f32)
                nc.vector.memset(zz[:], 0.0)
                nc.scalar.dma_start(loss_d.ap(), zz[:])

    nc.compile()
    _BUILD_CACHE[key] = nc
    return nc


def prep_inputs(cfg, input_ids, labels, embed, W):
    """Host-side sharding/layout prep. Returns (in_maps, denom)."""
    import ml_dtypes

    B, S, V, H = cfg["B"], cfg["S"], cfg["V"], cfg["H"]
    n_cores = cfg["n_cores"]
    T = B * S
    VS = V // n_cores
    MB = T // P
    IGNORE_INDEX = -100

    ids = np.ascontiguousarray(
        np.asarray(input_ids).reshape(T).astype(np.int32).reshape(MB, P).T
    )

    lab = np.asarray(labels).reshape(B, S)
    labshift = np.full((B, S), -3.0e9, np.float32)
    nxt = lab[:, 1:]
    valid = nxt != IGNORE_INDEX
    labshift[:, :-1] = np.where(valid, nxt.astype(np.float32), -3.0e9)
    denom = max(int(valid.sum()), 1)
    labflat = labshift.reshape(T)

    wm = np.zeros((B, S), np.float32)
    wm[:, :-1] = valid.astype(np.float32) / denom
    wm_arr = np.ascontiguousarray(wm.reshape(T).reshape(MB, P).T)

    embed_bf = np.asarray(embed, np.float32).astype(ml_dtypes.bfloat16)

    VS = V // n_cores
    NT = cfg["NT"]
    KC = H // P
    NQ = 4 if VS % (4 * NT) == 0 else 1
    QV = VS // NQ

    in_maps = []
    for c in range(n_cores):
        v0 = c * VS
        # wt[p, q, k, v] = W^T[k*128+p, q*QV+v] = W[v0+q*QV+v, k*128+p]
        wt_t = np.asarray(W, np.float32)[v0 : v0 + VS].T.astype(ml_dtypes.bfloat16)
        wt_c = np.ascontiguousarray(
            wt_t.reshape(KC, P, NQ, QV).transpose(1, 2, 0, 3)
        )
        labrel_c = np.ascontiguousarray(
            (labflat - np.float32(v0)).reshape(MB, P).T.astype(np.float32)
        )
        in_maps.append(
            {
                "embed": embed_bf,
                "wt": wt_c,
                "ids": ids,
                "labrel": labrel_c,
                "wmask": wm_arr,
            }
        )
    return in_maps, {"denom": denom, "wmask": wm_arr}


def assemble_outputs(cfg, results, extras):
    """Combine per-core outputs into (loss, logits)."""
    B, S, V = cfg["B"], cfg["S"], cfg["V"]
    n_cores = cfg["n_cores"]
    T = B * S
    MB = T // P
    logits = np.concatenate(
        [np.asarray(results[c]["logits"], np.float32) for c in range(n_cores)], axis=1
    ).reshape(B, S, V)
    if cfg["use_collective"]:
        loss = np.float32(np.asarray(results[0]["loss"]).reshape(-1)[0])
    else:
        # final 8-way sum of the per-shard [sumexp, label-logit] stats
        S_sum = np.zeros((P, MB), np.float64)
        T_sum = np.zeros((P, MB), np.float64)
        for c in range(n_cores):
            st = np.asarray(results[c]["stats"], np.float64)
            S_sum += st[:, 0:MB]
            T_sum += st[:, MB:]
        valid = extras["wmask"] > 0
        loss = np.float32(
            float(((np.log(S_sum) - T_sum) * valid).sum() / extras["denom"])
        )
    return loss, logits


def run_on_hw(cfg, in_maps, trace=False, **kw):
    from concourse import bass_utils

    nc = build_bass(cfg)
    res = bass_utils.run_bass_kernel_spmd(
        nc, in_maps, core_ids=list(range(cfg["n_cores"])), trace=trace, **kw
    )
    return res


def kernel(input_ids, labels, embed, W):
    cfg = dict(FULL_CFG)
    in_maps, denom = prep_inputs(cfg, input_ids, labels, embed, W)
    res = run_on_hw(cfg, in_maps, trace=False)
    return assemble_outputs(cfg, res.results, denom)
